# revision 1
# baseline (speedup 1.0000x reference)
"""Cross multihead attention (global/local masked head groups) on 8 trn2 cores.

Sharding: core c -> (batch b = c//2, head-group g = c%2).
  g=0: heads 0-7  masked by key_padding_mask[b]
  g=1: heads 8-15 masked by local_mask[b]
Each core computes its group's partial output  (attn_out_g @ Wo[:, g*512:(g+1)*512].T)
of shape [T, E]; the host sums the two partials per batch and adds bo.

On-chip layout ("transposed scores" orientation - zero on-chip transposes):
  qT, kT   : [512(j), 1024(t|s)]  feature-major (j = head*64 + d)
  v        : [1024(s), 8*65]      natural, per-head 65-col stripes [v_h | ones]
  scoresT  : [s, t] tiles; mask folded into Exp bias (per-partition = per-s)
  softmax  : no max-subtraction (scores ~ N(0,1)); denominators from the
             ones column of the augmented v matmul; normalization deferred
             to a per-head [64, 512] multiply with a partition-broadcast
             reciprocal row.
  attnT    : [512(j), 1024(t)] -> out = attnT.T @ woT accumulated over j-tiles.
"""

import os
import sys

sys.path.insert(0, "/opt/trn_rl_repo")

import numpy as np

import concourse.bass as bass
import concourse.mybir as mybir
from concourse.tile import TileContext

B, T, S, E, H = 4, 1024, 1024, 1024, 16
DH = E // H            # 64
HH = H // 2            # 8 heads per group
G = HH * DH            # 512 features per group
SCALING = DH ** -0.5
NEG = -30000.0         # exp(x + NEG) == 0.0 in fp32, no LUT edge cases

F32 = mybir.dt.float32
BF = mybir.dt.bfloat16   # tensor-engine operand dtype (1 cyc/row)


def _mm(ap):
    return ap


def _split_waits(nc):
    """TPB ISA structs hold one sem-wait slot. Tile can emit >1 wait per
    instruction (walrus: 'Too many sync wait commands'); hoist all but the
    last wait onto single-wait NOPs on the same engine, inserted just
    before. Timing is unchanged - the waits would have blocked anyway."""
    k = 0
    for f in nc.m.functions:
        for blk in f.blocks:
            new = []
            for inst in blk.instructions:
                si = inst.sync_info
                w = list(si.on_wait) if si else []
                if len(w) > 1:
                    for wait in w[:-1]:
                        nop = mybir.InstNoOp(name=f"nopw-{k}", ins=[], outs=[])
                        k += 1
                        nop.engine = inst.engine
                        nop.sync_info = mybir.SyncInfo(on_wait=[wait], on_update=[])
                        new.append(nop)
                    inst.sync_info = mybir.SyncInfo(
                        on_wait=[w[-1]], on_update=list(si.on_update)
                    )
                new.append(inst)
            blk.instructions = new
    return nc


def build_nc(split=True, phase='all'):
    nc = bass.Bass()

    xqT = nc.dram_tensor("xqT", [E, T], BF, kind="ExternalInput")
    xkT = nc.dram_tensor("xkT", [E, S], BF, kind="ExternalInput")
    xvT = nc.dram_tensor("xvT", [E, S], BF, kind="ExternalInput")
    wqT = nc.dram_tensor("wqT", [E, G], BF, kind="ExternalInput")
    wkT = nc.dram_tensor("wkT", [E, G], BF, kind="ExternalInput")
    wvT = nc.dram_tensor("wvT", [E, G], BF, kind="ExternalInput")
    woT = nc.dram_tensor("woT", [G, E], BF, kind="ExternalInput")
    mb = nc.dram_tensor("mb", [128, 8], F32, kind="ExternalInput")    # [-30000|0] per s
    bqc = nc.dram_tensor("bqc", [128, 4], F32, kind="ExternalInput")  # bq per j-tile col
    bkc = nc.dram_tensor("bkc", [128, 4], F32, kind="ExternalInput")
    bvr = nc.dram_tensor("bvr", [1, G], BF, kind="ExternalInput")    # bv as row
    out = nc.dram_tensor("out", [T, E], F32, kind="ExternalOutput")

    ET, ST, TT = E // 128, S // 128, T // 128   # 8, 8, 8
    JT = G // 128                               # 4 j-tiles
    NC = 512                                    # moving-operand chunk
    TC = T // NC                                # 2 t-chunks

    with TileContext(nc) as tc:
        with (
            tc.tile_pool(name="const", bufs=1) as pc,
            tc.tile_pool(name="persist", bufs=1) as pp,
            tc.tile_pool(name="xin", bufs=ET) as px,
            tc.tile_pool(name="win", bufs=ET) as pw,
            tc.tile_pool(name="exp", bufs=2 * ST) as pe,
            tc.tile_pool(name="outsb", bufs=3) as po,
            tc.tile_pool(name="small", bufs=4) as psm,
            tc.tile_pool(name="psg", bufs=2, space="PSUM") as ppsg,
            tc.tile_pool(name="pssc", bufs=4, space="PSUM") as ppsc,
            tc.tile_pool(name="psav", bufs=2, space="PSUM") as ppsav,
        ):
            # ---- constants ----
            mb_sb = pc.tile([128, 8], F32, name="mb_sb")
            nc.sync.dma_start(out=mb_sb[:], in_=mb[:])
            bq_sb = pc.tile([128, 4], F32, name="bq_sb")
            nc.sync.dma_start(out=bq_sb[:], in_=bqc[:])
            bk_sb = pc.tile([128, 4], F32, name="bk_sb")
            nc.sync.dma_start(out=bk_sb[:], in_=bkc[:])
            bv_sb = pc.tile([1, G], BF, name="bv_sb")
            nc.sync.dma_start(out=bv_sb[:], in_=bvr[:])
            ones_sb = pc.tile([2, 128], BF, name="ones_sb")
            nc.gpsimd.memset(ones_sb[:], 1.0)

            # ---- persistent activations ----
            qT_sb = [pp.tile([128, T], BF, name=f"qT{r}") for r in range(JT)]
            kT_sb = [pp.tile([128, S], BF, name=f"kT{r}") for r in range(JT)]
            v_sb = [pp.tile([128, HH * (DH + 1)], BF, name=f"v{st}") for st in range(ST)]
            aT_sb = [pp.tile([128, T], BF, name=f"aT{r}") for r in range(JT)]
            woT_sb = [pp.tile([128, E], BF, name=f"woT{r}") for r in range(JT)]

            # ---- q/k projections: out[j,t] = sum_e W.T[e,j] X.T[e,t] (+ bias) ----
            for pi, (xdr, wdr, dst, bias) in enumerate((
                (xqT, wqT, qT_sb, bq_sb),
                (xkT, wkT, kT_sb, bk_sb),
            )):
                xt = [px.tile([128, T], BF, tag=f"xe{pi}", name=f"xe{pi}_{et}") for et in range(ET)]
                wt = [pw.tile([128, G], BF, tag=f"we{pi}", name=f"we{pi}_{et}") for et in range(ET)]
                for et in range(ET):
                    nc.sync.dma_start(out=xt[et][:], in_=xdr[et * 128:(et + 1) * 128, :])
                    nc.sync.dma_start(out=wt[et][:], in_=wdr[et * 128:(et + 1) * 128, :])
                for r in range(JT):
                    for c2 in range(TC):
                        ps = ppsg.tile([128, NC], F32, tag="psg", name="ps_proj")
                        for et in range(ET):
                            nc.tensor.matmul(
                                ps[:],
                                lhsT=_mm(wt[et][:, r * 128:(r + 1) * 128]),
                                rhs=_mm(xt[et][:, c2 * NC:(c2 + 1) * NC]),
                                start=(et == 0), stop=(et == ET - 1),
                            )
                        nc.vector.tensor_scalar_add(
                            dst[r][:, c2 * NC:(c2 + 1) * NC], ps[:], bias[:, r:r + 1]
                        )

            # ---- v projection: v[s,d] = sum_e X.T[e,s] Wv.T[e,d] + bv ----
            xt = [px.tile([128, S], BF, tag="xev", name=f"xve{et}") for et in range(ET)]
            wt = [pw.tile([128, G], BF, tag="wev", name=f"wve{et}") for et in range(ET)]
            for et in range(ET):
                nc.sync.dma_start(out=xt[et][:], in_=xvT[et * 128:(et + 1) * 128, :])
                nc.sync.dma_start(out=wt[et][:], in_=wvT[et * 128:(et + 1) * 128, :])
            for st in range(ST):
                ps = ppsg.tile([128, G], F32, tag="psg", name="ps_v")
                for et in range(ET):
                    nc.tensor.matmul(
                        ps[:],
                        lhsT=_mm(xt[et][:, st * 128:(st + 1) * 128]),
                        rhs=_mm(wt[et][:]),
                        start=(et == 0), stop=False,
                    )
                nc.tensor.matmul(  # += ones[1,128].T @ bv[1,512]
                    ps[:], lhsT=_mm(ones_sb[0:1, :]), rhs=_mm(bv_sb[:]),
                    start=False, stop=True,
                )
                # scatter [128, 8, 64] into 65-col stripes; stripe col 64 <- 1.0
                v3 = v_sb[st][:].rearrange("p (h x) -> p h x", x=DH + 1)
                nc.vector.tensor_copy(
                    v3[:, :, 0:DH], ps[:].rearrange("p (h x) -> p h x", x=DH)
                )
                nc.gpsimd.memset(v3[:, :, DH:DH + 1], 1.0)

            for r in range(JT):
                nc.sync.dma_start(out=woT_sb[r][:], in_=woT[r * 128:(r + 1) * 128, :])

            if phase == 'proj':
                for r in range(JT):
                    ot = po.tile([128, T], F32, tag="otp", name=f"otp{r}")
                    nc.vector.tensor_copy(ot[:], qT_sb[r][:])
                    nc.sync.dma_start(out=out[r * 128:(r + 1) * 128, :], in_=ot[:])
                    ot2 = po.tile([128, T], F32, tag="otp", name=f"otp2{r}")
                    nc.vector.tensor_copy(ot2[:], kT_sb[r][:])
                    nc.sync.dma_start(out=out[512 + r * 128:512 + (r + 1) * 128, :], in_=ot2[:])

            # ---- attention ----
            for c in range(TC if phase == 'all' else 0):
                tsl = slice(c * NC, (c + 1) * NC)
                for hp in range(HH // 2):
                    pair = (2 * hp, 2 * hp + 1)
                    expT = {h: [pe.tile([128, NC], BF, tag="exp", name=f"exp_h{h}_s{st}") for st in range(ST)]
                            for h in pair}
                    for st in range(ST):
                        for h in pair:
                            r, po_ = h // 2, (h % 2) * DH
                            ps_s = ppsc.tile([128, NC], F32, tag="sc", name="ps_s")
                            nc.tensor.matmul(
                                ps_s[:],
                                lhsT=_mm(kT_sb[r][po_:po_ + DH, st * 128:(st + 1) * 128]),
                                rhs=_mm(qT_sb[r][po_:po_ + DH, tsl]),
                                start=True, stop=True,
                            )
                            nc.scalar.activation(
                                expT[h][st][:], ps_s[:],
                                mybir.ActivationFunctionType.Exp,
                                bias=mb_sb[:, st:st + 1], scale=SCALING,
                            )
                    for h in pair:
                        r, po_ = h // 2, (h % 2) * DH
                        ps_o = ppsav.tile([DH + 1, NC], F32, tag="av", name="ps_o")
                        for st in range(ST):
                            nc.tensor.matmul(
                                ps_o[:],
                                lhsT=_mm(v_sb[st][:, h * (DH + 1):(h + 1) * (DH + 1)]),
                                rhs=_mm(expT[h][st][:]),
                                start=(st == 0), stop=(st == ST - 1),
                            )
                        rec = psm.tile([1, NC], F32, tag="rec", name="rec")
                        nc.vector.reciprocal(rec[:], ps_o[DH:DH + 1, :])
                        # broadcast rec across 64 partitions at ~fp32 precision:
                        # hi = bf16(rec), lo = bf16(rec - hi);  ones[2,64].T @ [hi;lo]
                        # sums hi+lo in fp32 PSUM.
                        rhi = psm.tile([1, NC], BF, tag="rhi", name="rhi")
                        nc.vector.tensor_copy(rhi[:], rec[:])
                        rlo = psm.tile([1, NC], BF, tag="rlo", name="rlo")
                        nc.vector.tensor_sub(rlo[:], rec[:], rhi[:])
                        ps_b = ppsc.tile([DH, NC], F32, tag="sc", name="ps_b")
                        nc.tensor.matmul(ps_b[:], lhsT=ones_sb[0:1, 0:DH],
                                         rhs=rhi[:], start=True, stop=False)
                        nc.tensor.matmul(ps_b[:], lhsT=ones_sb[0:1, 0:DH],
                                         rhs=rlo[:], start=False, stop=True)
                        rb = psm.tile([DH, NC], F32, tag="rb", name="rb")
                        nc.vector.tensor_copy(rb[:], ps_b[:])
                        nc.vector.tensor_mul(
                            aT_sb[r][po_:po_ + DH, tsl],
                            ps_o[0:DH, :],
                            rb[:],
                        )
                # ---- output projection for this chunk's t-tiles ----
                for tt in range(c * 4, c * 4 + 4):
                    for oc in range(2):
                        ps_u = ppsg.tile([128, NC], F32, tag="psg", name="ps_u")
                        for r in range(JT):
                            nc.tensor.matmul(
                                ps_u[:],
                                lhsT=_mm(aT_sb[r][:, tt * 128:(tt + 1) * 128]),
                                rhs=_mm(woT_sb[r][:, oc * NC:(oc + 1) * NC]),
                                start=(r == 0), stop=(r == JT - 1),
                            )
                        ot = po.tile([128, NC], F32, tag="ot", name="ot")
                        nc.vector.tensor_copy(ot[:], ps_u[:])
                        nc.sync.dma_start(
                            out=out[tt * 128:(tt + 1) * 128, oc * NC:(oc + 1) * NC],
                            in_=ot[:],
                        )
    return _split_waits(nc) if split else nc


_NC_CACHE = None


def _get_nc():
    global _NC_CACHE
    if _NC_CACHE is None:
        _NC_CACHE = build_nc()
    return _NC_CACHE


def make_in_maps(query, key, value, key_padding_mask, local_mask,
                 Wq, bq, Wk, bk, Wv, bv, Wo, bo):
    import ml_dtypes
    f = np.float32
    bf = ml_dtypes.bfloat16
    in_maps = []
    for c in range(8):
        b, g = c // 2, c % 2
        gs = slice(g * G, (g + 1) * G)
        mask = (key_padding_mask if g == 0 else local_mask)[b]
        mbias = np.where(mask, NEG, 0.0).astype(f).reshape(8, 128).T  # [128, 8]
        in_maps.append({
            "xqT": np.ascontiguousarray(query[b].T, dtype=bf),
            "xkT": np.ascontiguousarray(key[b].T, dtype=bf),
            "xvT": np.ascontiguousarray(value[b].T, dtype=bf),
            "wqT": np.ascontiguousarray(Wq[gs, :].T, dtype=bf),
            "wkT": np.ascontiguousarray(Wk[gs, :].T, dtype=bf),
            "wvT": np.ascontiguousarray(Wv[gs, :].T, dtype=bf),
            "woT": np.ascontiguousarray(Wo[:, gs].T, dtype=bf),
            "mb": np.ascontiguousarray(mbias),
            "bqc": np.ascontiguousarray(bq[gs].astype(f).reshape(4, 128).T),
            "bkc": np.ascontiguousarray(bk[gs].astype(f).reshape(4, 128).T),
            "bvr": np.ascontiguousarray(bv[gs].astype(bf).reshape(1, G)),
        })
    return in_maps


def kernel(query, key, value, key_padding_mask, local_mask,
           Wq, bq, Wk, bk, Wv, bv, Wo, bo, _trace=False, _tmpdir=None):
    from concourse.bass_utils import run_bass_kernel_spmd

    nc = _get_nc()
    in_maps = make_in_maps(query, key, value, key_padding_mask, local_mask,
                           Wq, bq, Wk, bk, Wv, bv, Wo, bo)
    try:
        res = run_bass_kernel_spmd(nc, in_maps, list(range(8)),
                                   trace=_trace, tmpdir=_tmpdir)
    except Exception:
        # transient device/transport failures have been observed on the
        # axon path; one fresh attempt is cheap relative to a hard fail
        res = run_bass_kernel_spmd(nc, in_maps, list(range(8)),
                                   trace=_trace, tmpdir=_tmpdir)
    outs = [np.asarray(r["out"]) for r in res.results]
    full = np.stack([outs[2 * b] + outs[2 * b + 1] for b in range(B)])
    full += np.asarray(bo, dtype=np.float32)
    if _trace:
        kernel._last_exec_time_ns = res.exec_time_ns
        kernel._last_profile = res.profile_json
    return full.astype(np.float32)



# revision 2
# speedup vs baseline: 1.4676x; 1.4676x over previous
"""Cross multihead attention (global/local masked head groups) on 8 trn2 cores.

Sharding: core c -> (batch b = c//2, head-group g = c%2).
  g=0: heads 0-7  masked by key_padding_mask[b]
  g=1: heads 8-15 masked by local_mask[b]
Each core computes its group's partial output  (attn_out_g @ Wo[:, g*512:(g+1)*512].T)
of shape [T, E]; the host sums the two partials per batch and adds bo.

On-chip design (v2 - "natural av" orientation, head-pair pipeline):
  qT, kT   : [128(j), 1024(t|s)] per j-tile r, feature-major (j = head*64 + d)
  v        : [1024(s), 8*65]     natural, per-head 65-col stripes [v_h | ones]
  scoresT  : [s-tile 128, T 1024] psum (2 banks), mask folded into Exp bias
             (per-partition = per-s); exp -> bf16 sbuf tiles expT[h][st].
  av       : natural orientation. out[t, d] psum [128, 130] per (head-pair,
             t-tile): stationary = expT 128x128 block, moving = v stripe
             [128, 65] -> 65-cycle matmuls; col 64 of each stripe is the
             softmax denominator (ones column of v).
  norm     : reciprocal of the denominator columns (per-partition scalar) +
             tensor_scalar_mul -> a_nat [t-tile][128, 512(j)] bf16.
  aT       : DMA-engine transpose (dma_start_transpose) of 128x128 blocks of
             a_nat -> aT[r][128, T]; out = aT.T @ woT accumulated over r.
  pipeline : per head-pair r: q/k proj r, scores+exp(r) interleaved with
             av(r-1); v-proj fills the PE during the first period.
"""

import os
import sys

sys.path.insert(0, "/opt/trn_rl_repo")

import numpy as np

import concourse.bass as bass
import concourse.mybir as mybir
from concourse.tile import TileContext

B, T, S, E, H = 4, 1024, 1024, 1024, 16
DH = E // H            # 64
HH = H // 2            # 8 heads per group
G = HH * DH            # 512 features per group
SCALING = DH ** -0.5
NEG = -30000.0         # exp(x + NEG) == 0.0 in fp32, no LUT edge cases

F32 = mybir.dt.float32
BF = mybir.dt.bfloat16   # tensor-engine operand dtype (1 cyc/row)


def _split_waits(nc):
    """TPB ISA structs hold one sem-wait slot. Tile can emit >1 wait per
    instruction (walrus: 'Too many sync wait commands'); hoist all but the
    last wait onto single-wait NOPs on the same engine, inserted just
    before. Timing is unchanged - the waits would have blocked anyway."""
    k = 0
    for f in nc.m.functions:
        for blk in f.blocks:
            new = []
            for inst in blk.instructions:
                si = inst.sync_info
                w = list(si.on_wait) if si else []
                if len(w) > 1:
                    for wait in w[:-1]:
                        nop = mybir.InstNoOp(name=f"nopw-{k}", ins=[], outs=[])
                        k += 1
                        nop.engine = inst.engine
                        nop.sync_info = mybir.SyncInfo(on_wait=[wait], on_update=[])
                        new.append(nop)
                    inst.sync_info = mybir.SyncInfo(
                        on_wait=[w[-1]], on_update=list(si.on_update)
                    )
                new.append(inst)
            blk.instructions = new
    return nc


def build_nc(split=True):
    nc = bass.Bass()

    xqT = nc.dram_tensor("xqT", [E, T], BF, kind="ExternalInput")
    xkT = nc.dram_tensor("xkT", [E, S], BF, kind="ExternalInput")
    xvT = nc.dram_tensor("xvT", [E, S], BF, kind="ExternalInput")
    wqT = nc.dram_tensor("wqT", [E, G], BF, kind="ExternalInput")
    wkT = nc.dram_tensor("wkT", [E, G], BF, kind="ExternalInput")
    wvT = nc.dram_tensor("wvT", [E, G], BF, kind="ExternalInput")
    woT = nc.dram_tensor("woT", [G, E], BF, kind="ExternalInput")
    mb = nc.dram_tensor("mb", [128, 8], F32, kind="ExternalInput")    # [-30000|0] per s
    bqc = nc.dram_tensor("bqc", [128, 4], F32, kind="ExternalInput")  # bq per j-tile col
    bkc = nc.dram_tensor("bkc", [128, 4], F32, kind="ExternalInput")
    bvb = nc.dram_tensor("bvb", [128, G], BF, kind="ExternalInput")   # bv bcast rows
    out = nc.dram_tensor("out", [T, E], F32, kind="ExternalOutput")

    ET, ST, TT = E // 128, S // 128, T // 128   # 8, 8, 8
    JT = G // 128                               # 4 j-tiles == head pairs
    VW = DH + 1                                 # 65-col v stripe

    with TileContext(nc) as tc:
        with (
            tc.tile_pool(name="const", bufs=1) as pc,
            tc.tile_pool(name="persist", bufs=1) as pp,
            tc.tile_pool(name="xin", bufs=ET) as px,
            tc.tile_pool(name="win", bufs=ET) as pw,
            tc.tile_pool(name="exp", bufs=4 * ST) as pe,
            tc.tile_pool(name="rec", bufs=4) as prc,
            tc.tile_pool(name="outsb", bufs=4) as po,
            tc.tile_pool(name="psg", bufs=2, space="PSUM") as ppsg,
            tc.tile_pool(name="pssc", bufs=2, space="PSUM") as ppsc,
            tc.tile_pool(name="psav", bufs=2, space="PSUM") as ppsav,
        ):
            # ---- constants (SP channel, first: tiny + needed early-ish) ----
            mb_sb = pc.tile([128, 8], F32, name="mb_sb")
            nc.sync.dma_start(out=mb_sb[:], in_=mb[:])
            bq_sb = pc.tile([128, 4], F32, name="bq_sb")
            nc.sync.dma_start(out=bq_sb[:], in_=bqc[:])
            bk_sb = pc.tile([128, 4], F32, name="bk_sb")
            nc.sync.dma_start(out=bk_sb[:], in_=bkc[:])
            bvb_sb = pc.tile([128, G], BF, name="bvb_sb")
            nc.sync.dma_start(out=bvb_sb[:], in_=bvb[:])

            # ---- persistent activations ----
            qT_sb = [pp.tile([128, T], BF, name=f"qT{r}") for r in range(JT)]
            kT_sb = [pp.tile([128, S], BF, name=f"kT{r}") for r in range(JT)]
            v_sb = [pp.tile([128, HH * VW], BF, name=f"v{st}") for st in range(ST)]
            an_sb = [pp.tile([128, G], BF, name=f"an{tt}") for tt in range(TT)]
            aT_sb = [pp.tile([128, T], BF, name=f"aT{r}") for r in range(JT)]
            woT_sb = [pp.tile([128, E], BF, name=f"woT{r}") for r in range(JT)]

            # ---- input DMAs across three channels ----
            # SP:   xq, then wv, xv, woT        (q first for pipeline start)
            # Act:  wq, wk, bvb?, (exp work starts ~12us, DMAs done ~8us)
            # Pool: xk (swdge)
            xq = [px.tile([128, T], BF, tag="xq", name=f"xq{et}") for et in range(ET)]
            wq = [pw.tile([128, G], BF, tag="wq", name=f"wq{et}") for et in range(ET)]
            xk = [px.tile([128, S], BF, tag="xk", name=f"xk{et}") for et in range(ET)]
            wk = [pw.tile([128, G], BF, tag="wk", name=f"wk{et}") for et in range(ET)]
            xv = [px.tile([128, S], BF, tag="xv", name=f"xv{et}") for et in range(ET)]
            wv = [pw.tile([128, G], BF, tag="wv", name=f"wv{et}") for et in range(ET)]

            for et in range(ET):
                nc.scalar.dma_start(out=wq[et][:], in_=wqT[et * 128:(et + 1) * 128, :])
            for et in range(ET):
                nc.scalar.dma_start(out=wk[et][:], in_=wkT[et * 128:(et + 1) * 128, :])
            for et in range(ET):
                nc.sync.dma_start(out=xq[et][:], in_=xqT[et * 128:(et + 1) * 128, :])
            for et in range(ET):
                nc.gpsimd.dma_start(out=xk[et][:], in_=xkT[et * 128:(et + 1) * 128, :])
            for et in range(ET):
                nc.sync.dma_start(out=wv[et][:], in_=wvT[et * 128:(et + 1) * 128, :])
            for et in range(ET):
                nc.sync.dma_start(out=xv[et][:], in_=xvT[et * 128:(et + 1) * 128, :])
            for r in range(JT):
                nc.scalar.dma_start(out=woT_sb[r][:], in_=woT[r * 128:(r + 1) * 128, :])

            # v stripe ones columns (denominator accumulators)
            for st in range(ST):
                v3 = v_sb[st][:].rearrange("p (h x) -> p h x", x=VW)
                nc.gpsimd.memset(v3[:, :, DH:DH + 1], 1.0)

            # ---- emission helpers ----
            def proj_qk(which, r):
                """q/k projection for j-tile r: psum [128, 512] per t-half,
                bias add (per-partition) -> qT/kT sbuf."""
                xt, wt, dst, bias = (
                    (xq, wq, qT_sb, bq_sb) if which == "q" else (xk, wk, kT_sb, bk_sb)
                )
                for c2 in range(2):
                    ps = ppsg.tile([128, 512], F32, tag="psg", name=f"ps_{which}{r}")
                    for et in range(ET):
                        nc.tensor.matmul(
                            ps[:],
                            lhsT=wt[et][:, r * 128:(r + 1) * 128],
                            rhs=xt[et][:, c2 * 512:(c2 + 1) * 512],
                            start=(et == 0), stop=(et == ET - 1),
                        )
                    nc.vector.tensor_scalar_add(
                        dst[r][:, c2 * 512:(c2 + 1) * 512], ps[:], bias[:, r:r + 1]
                    )

            def proj_v(st):
                """v projection for s-tile st -> 65-col stripes with bias."""
                ps = ppsg.tile([128, G], F32, tag="psg", name=f"ps_v{st}")
                for et in range(ET):
                    nc.tensor.matmul(
                        ps[:],
                        lhsT=xv[et][:, st * 128:(st + 1) * 128],
                        rhs=wv[et][:],
                        start=(et == 0), stop=(et == ET - 1),
                    )
                v3 = v_sb[st][:].rearrange("p (h x) -> p h x", x=VW)
                nc.vector.tensor_tensor(
                    v3[:, :, 0:DH],
                    ps[:].rearrange("p (h x) -> p h x", x=DH),
                    bvb_sb[:].rearrange("p (h x) -> p h x", x=DH),
                    mybir.AluOpType.add,
                )

            expT = {}   # (h, st) -> sbuf tile [128, T] bf16

            def scores_exp(h, st):
                """scoresT psum [s-tile, T] (2 banks) -> exp -> bf16 sbuf."""
                r, po_ = h // 2, (h % 2) * DH
                ps_s = ppsc.tile([128, T], F32, tag="sc", name=f"ps_s{h}_{st}")
                for c2 in range(2):
                    nc.tensor.matmul(
                        ps_s[:, c2 * 512:(c2 + 1) * 512],
                        lhsT=kT_sb[r][po_:po_ + DH, st * 128:(st + 1) * 128],
                        rhs=qT_sb[r][po_:po_ + DH, c2 * 512:(c2 + 1) * 512],
                        start=True, stop=True,
                    )
                e = pe.tile([128, T], BF, tag="exp", name=f"e{h}_{st}")
                nc.scalar.activation(
                    e[:], ps_s[:],
                    mybir.ActivationFunctionType.Exp,
                    bias=mb_sb[:, st:st + 1], scale=SCALING,
                )
                expT[(h, st)] = e

            def av_block(r, tt):
                """natural-orientation attn@v for head pair r, t-tile tt:
                psum [t 128, 130] stripes -> reciprocal + per-partition
                normalize -> a_nat bf16; then DMA-transpose into aT[r]."""
                ps_o = ppsav.tile([128, 2 * VW], F32, tag="av", name=f"ps_o{r}_{tt}")
                for hh in range(2):
                    h = 2 * r + hh
                    for st in range(ST):
                        nc.tensor.matmul(
                            ps_o[:, hh * VW:(hh + 1) * VW],
                            lhsT=expT[(h, st)][:, tt * 128:(tt + 1) * 128],
                            rhs=v_sb[st][:, h * VW:(h + 1) * VW],
                            start=(st == 0), stop=(st == ST - 1),
                        )
                rec = prc.tile([128, 2], F32, tag="rec", name=f"rec{r}_{tt}")
                nc.vector.reciprocal(rec[:], ps_o[:, DH::VW])
                for hh in range(2):
                    nc.vector.tensor_scalar_mul(
                        an_sb[tt][:, r * 128 + hh * DH:r * 128 + (hh + 1) * DH],
                        ps_o[:, hh * VW:hh * VW + DH],
                        rec[:, hh:hh + 1],
                    )
                nc.sync.dma_start_transpose(
                    aT_sb[r][:, tt * 128:(tt + 1) * 128],
                    an_sb[tt][:, r * 128:(r + 1) * 128],
                )

            def outproj(tt):
                """out[t-tile] = sum_r aT[r].T @ woT[r]; psum -> sbuf -> HBM."""
                for oc in range(2):
                    ps_u = ppsg.tile([128, 512], F32, tag="psg", name=f"ps_u{tt}_{oc}")
                    for r in range(JT):
                        nc.tensor.matmul(
                            ps_u[:],
                            lhsT=aT_sb[r][:, tt * 128:(tt + 1) * 128],
                            rhs=woT_sb[r][:, oc * 512:(oc + 1) * 512],
                            start=(r == 0), stop=(r == JT - 1),
                        )
                    ot = po.tile([128, 512], F32, tag="ot", name=f"ot{tt}_{oc}")
                    if (tt + oc) % 2 == 0:
                        nc.vector.tensor_copy(ot[:], ps_u[:])
                    else:
                        nc.scalar.copy(ot[:], ps_u[:])
                    eng = nc.sync if oc == 0 else nc.scalar
                    eng.dma_start(
                        out=out[tt * 128:(tt + 1) * 128, oc * 512:(oc + 1) * 512],
                        in_=ot[:],
                    )

            # ---- pipelined emission ----
            # period r (r = 0..3): q/k proj r, scores+exp for heads 2r/2r+1
            # interleaved with av blocks of head pair r-1; period 0 uses the
            # v projection as PE filler while the exp pipeline fills.
            for r in range(JT):
                proj_qk("q", r)
                proj_qk("k", r)
                for st in range(ST):
                    scores_exp(2 * r, st)
                    scores_exp(2 * r + 1, st)
                    if r == 0:
                        proj_v(st)
                    else:
                        av_block(r - 1, tt=st)
            # drain: av of last head pair, then output projection per t-tile
            for tt in range(TT):
                av_block(JT - 1, tt)
                outproj(tt)

    return _split_waits(nc) if split else nc


_NC_CACHE = None


def _get_nc():
    global _NC_CACHE
    if _NC_CACHE is None:
        _NC_CACHE = build_nc()
    return _NC_CACHE


def make_in_maps(query, key, value, key_padding_mask, local_mask,
                 Wq, bq, Wk, bk, Wv, bv, Wo, bo):
    import ml_dtypes
    f = np.float32
    bf = ml_dtypes.bfloat16
    in_maps = []
    for c in range(8):
        b, g = c // 2, c % 2
        gs = slice(g * G, (g + 1) * G)
        mask = (key_padding_mask if g == 0 else local_mask)[b]
        mbias = np.where(mask, NEG, 0.0).astype(f).reshape(8, 128).T  # [128, 8]
        in_maps.append({
            "xqT": np.ascontiguousarray(query[b].T, dtype=bf),
            "xkT": np.ascontiguousarray(key[b].T, dtype=bf),
            "xvT": np.ascontiguousarray(value[b].T, dtype=bf),
            "wqT": np.ascontiguousarray(Wq[gs, :].T, dtype=bf),
            "wkT": np.ascontiguousarray(Wk[gs, :].T, dtype=bf),
            "wvT": np.ascontiguousarray(Wv[gs, :].T, dtype=bf),
            "woT": np.ascontiguousarray(Wo[:, gs].T, dtype=bf),
            "mb": np.ascontiguousarray(mbias),
            "bqc": np.ascontiguousarray(bq[gs].astype(f).reshape(4, 128).T),
            "bkc": np.ascontiguousarray(bk[gs].astype(f).reshape(4, 128).T),
            "bvb": np.ascontiguousarray(
                np.broadcast_to(bv[gs].astype(bf), (128, G))
            ),
        })
    return in_maps


def kernel(query, key, value, key_padding_mask, local_mask,
           Wq, bq, Wk, bk, Wv, bv, Wo, bo, _trace=False, _tmpdir=None):
    from concourse.bass_utils import run_bass_kernel_spmd

    nc = _get_nc()
    in_maps = make_in_maps(query, key, value, key_padding_mask, local_mask,
                           Wq, bq, Wk, bk, Wv, bv, Wo, bo)
    try:
        res = run_bass_kernel_spmd(nc, in_maps, list(range(8)),
                                   trace=_trace, tmpdir=_tmpdir)
    except Exception:
        # transient device/transport failures have been observed on the
        # axon path; one fresh attempt is cheap relative to a hard fail
        res = run_bass_kernel_spmd(nc, in_maps, list(range(8)),
                                   trace=_trace, tmpdir=_tmpdir)
    outs = [np.asarray(r["out"]) for r in res.results]
    full = np.stack([outs[2 * b] + outs[2 * b + 1] for b in range(B)])
    full += np.asarray(bo, dtype=np.float32)
    if _trace:
        kernel._last_exec_time_ns = res.exec_time_ns
        kernel._last_profile = res.profile_json
    return full.astype(np.float32)


# revision 4
# speedup vs baseline: 1.9077x; 1.2998x over previous
"""Cross multihead attention (global/local masked head groups) on 8 trn2 cores.

Sharding: core c -> (batch b = c//2, head-group g = c%2).
  g=0: heads 0-7  masked by key_padding_mask[b]
  g=1: heads 8-15 masked by local_mask[b]
Each core computes its group's partial output  (attn_out_g @ Wo[:, g*512:(g+1)*512].T)
of shape [T, E]; the host sums the two partials per batch and adds bo.

Sparsity: the masks knock out ~50% of key/value rows, and masked rows are
mathematically dead (softmax weight 0). The host gathers the unmasked kv
columns and pads to a static S2 = 640 (5 s-tiles; kept counts are ~500-550,
binomial(1024, 1/2), so 640 is an +8 sigma bound). Pad columns get the
-30000 exp bias, so the device result is bit-identical to the dense math.
If a pathological input ever exceeds 640 kept rows, kernel() falls back to
a dense S2 = 1024 build of the same code.

On-chip design ("natural av" orientation, head-pair pipeline):
  qT, kT   : [128(j), T|S2] per j-tile r, feature-major (j = head*64 + d)
  v        : [S2(s), 8*65]  natural, per-head 65-col stripes [v_h | ones]
  scoresT  : [s-tile 128, T 1024] psum (2 banks), mask/pad folded into the
             Exp bias (per-partition = per-s); exp -> bf16 sbuf expT[h][st].
  av       : natural orientation: out[t, d] psum [128, 2*65] per (head-pair,
             t-tile); stationary = expT 128x128 block, moving = v stripe
             [128, 65] -> 65-cycle matmuls; col 64 of each stripe is the
             softmax denominator (ones column of v).
  norm     : reciprocal of the denominator columns (per-partition scalar) +
             tensor_scalar_mul -> a_nat [t-tile][128, 512(j)] bf16.
  aT       : DMA-engine transpose (dma_start_transpose) of 128x128 blocks of
             a_nat -> aT[r][128, T]; out = aT.T @ woT accumulated over r.
  pipeline : per head-pair r: q/k proj r, then scores+exp(r) interleaved
             with av(r-1) blocks (ties the PSUM scores ring to the exp
             drain); v-proj fills the PE during period 0.
  DMA      : three channels - SP: consts, xq, xv, woT, aT transposes, half
             the output stores; Act: wq, wk, wv then exps + other output
             stores; Pool (swdge): xk. Output psum is copied to sbuf on DVE
             and the last tile is streamed out in 128-col pieces so the
             final store's latency is mostly hidden.
"""

import os
import sys

sys.path.insert(0, "/opt/trn_rl_repo")

import numpy as np

import concourse.bass as bass
import concourse.mybir as mybir
from concourse.tile import TileContext

B, T, S, E, H = 4, 1024, 1024, 1024, 16
DH = E // H            # 64
HH = H // 2            # 8 heads per group
G = HH * DH            # 512 features per group
SCALING = DH ** -0.5
NEG = -30000.0         # exp(x + NEG) == 0.0 in fp32, no LUT edge cases
SKEEP = 640            # padded gathered kv length (5 tiles of 128)

F32 = mybir.dt.float32
BF = mybir.dt.bfloat16   # tensor-engine operand dtype (1 cyc/row)


def _split_waits(nc):
    """TPB ISA structs hold one sem-wait slot. Tile can emit >1 wait per
    instruction (walrus: 'Too many sync wait commands'); hoist all but the
    last wait onto single-wait NOPs on the same engine, inserted just
    before. Timing is unchanged - the waits would have blocked anyway."""
    k = 0
    for f in nc.m.functions:
        for blk in f.blocks:
            new = []
            for inst in blk.instructions:
                si = inst.sync_info
                w = list(si.on_wait) if si else []
                if len(w) > 1:
                    for wait in w[:-1]:
                        nop = mybir.InstNoOp(name=f"nopw-{k}", ins=[], outs=[])
                        k += 1
                        nop.engine = inst.engine
                        nop.sync_info = mybir.SyncInfo(on_wait=[wait], on_update=[])
                        new.append(nop)
                    inst.sync_info = mybir.SyncInfo(
                        on_wait=[w[-1]], on_update=list(si.on_update)
                    )
                new.append(inst)
            blk.instructions = new
    return nc


def build_nc(split=True, s2=SKEEP):
    nc = bass.Bass()

    ST2 = s2 // 128                             # gathered kv s-tiles
    xqT = nc.dram_tensor("xqT", [E, T], BF, kind="ExternalInput")
    xkT = nc.dram_tensor("xkT", [E, s2], BF, kind="ExternalInput")
    xvT = nc.dram_tensor("xvT", [E, s2], BF, kind="ExternalInput")
    wqT = nc.dram_tensor("wqT", [E, G], BF, kind="ExternalInput")
    wkT = nc.dram_tensor("wkT", [E, G], BF, kind="ExternalInput")
    wvT = nc.dram_tensor("wvT", [E, G], BF, kind="ExternalInput")
    woT = nc.dram_tensor("woT", [G, E], BF, kind="ExternalInput")
    mb = nc.dram_tensor("mb", [128, ST2], F32, kind="ExternalInput")  # 0 | -30000 per s
    bqc = nc.dram_tensor("bqc", [128, 4], F32, kind="ExternalInput")  # bq per j-tile col
    bkc = nc.dram_tensor("bkc", [128, 4], F32, kind="ExternalInput")
    bvb = nc.dram_tensor("bvb", [128, G], BF, kind="ExternalInput")   # bv bcast rows
    out = nc.dram_tensor("out", [T, E], F32, kind="ExternalOutput")

    ET, TT = E // 128, T // 128                 # 8, 8
    JT = G // 128                               # 4 j-tiles == head pairs
    VW = DH + 1                                 # 65-col v stripe
    KC = s2 // 2                                # k-proj half width (<= 512)

    with TileContext(nc) as tc:
        with (
            tc.tile_pool(name="const", bufs=1) as pc,
            tc.tile_pool(name="persist", bufs=1) as pp,
            tc.tile_pool(name="xin", bufs=ET) as px,
            tc.tile_pool(name="win", bufs=ET) as pw,
            tc.tile_pool(name="exp", bufs=4 * ST2) as pe,
            tc.tile_pool(name="rec", bufs=4) as prc,
            tc.tile_pool(name="outsb", bufs=6) as po,
            tc.tile_pool(name="psg", bufs=2, space="PSUM") as ppsg,
            tc.tile_pool(name="pssc", bufs=2, space="PSUM") as ppsc,
            tc.tile_pool(name="psav", bufs=2, space="PSUM") as ppsav,
        ):
            # ---- persistent activations ----
            qT_sb = [pp.tile([128, T], BF, name=f"qT{r}") for r in range(JT)]
            kT_sb = [pp.tile([128, s2], BF, name=f"kT{r}") for r in range(JT)]
            v_sb = [pp.tile([128, HH * VW], BF, name=f"v{st}") for st in range(ST2)]
            an_sb = [pp.tile([128, G], BF, name=f"an{tt}") for tt in range(TT)]
            aT_sb = [pp.tile([128, T], BF, name=f"aT{r}") for r in range(JT)]
            woT_sb = [pp.tile([128, E], BF, name=f"woT{r}") for r in range(JT)]

            # ---- input DMAs across three channels ----
            xq = [px.tile([128, T], BF, tag="xq", name=f"xq{et}") for et in range(ET)]
            wq = [pw.tile([128, G], BF, tag="wq", name=f"wq{et}") for et in range(ET)]
            xk = [px.tile([128, s2], BF, tag="xk", name=f"xk{et}") for et in range(ET)]
            wk = [pw.tile([128, G], BF, tag="wk", name=f"wk{et}") for et in range(ET)]
            xv = [px.tile([128, s2], BF, tag="xv", name=f"xv{et}") for et in range(ET)]
            wv = [pw.tile([128, G], BF, tag="wv", name=f"wv{et}") for et in range(ET)]

            # Pool (swdge): consts then xk - ready early for k-proj r0
            mb_sb = pc.tile([128, ST2], F32, name="mb_sb")
            nc.gpsimd.dma_start(out=mb_sb[:], in_=mb[:])
            bq_sb = pc.tile([128, 4], F32, name="bq_sb")
            nc.gpsimd.dma_start(out=bq_sb[:], in_=bqc[:])
            bk_sb = pc.tile([128, 4], F32, name="bk_sb")
            nc.gpsimd.dma_start(out=bk_sb[:], in_=bkc[:])
            bvb_sb = pc.tile([128, G], BF, name="bvb_sb")
            nc.gpsimd.dma_start(out=bvb_sb[:], in_=bvb[:])
            for et in range(ET):
                nc.gpsimd.dma_start(out=xk[et][:], in_=xkT[et * 128:(et + 1) * 128, :])
            # Act: weights for q/k/v then nothing but exps until the drain
            for et in range(ET):
                nc.scalar.dma_start(out=wq[et][:], in_=wqT[et * 128:(et + 1) * 128, :])
            for et in range(ET):
                nc.scalar.dma_start(out=wk[et][:], in_=wkT[et * 128:(et + 1) * 128, :])
            for et in range(ET):
                nc.scalar.dma_start(out=wv[et][:], in_=wvT[et * 128:(et + 1) * 128, :])
            # SP: xq (pipeline start), xv, woT
            for et in range(ET):
                nc.sync.dma_start(out=xq[et][:], in_=xqT[et * 128:(et + 1) * 128, :])
            for et in range(ET):
                nc.sync.dma_start(out=xv[et][:], in_=xvT[et * 128:(et + 1) * 128, :])
            for r in range(JT):
                nc.sync.dma_start(out=woT_sb[r][:], in_=woT[r * 128:(r + 1) * 128, :])

            # v stripe ones columns (denominator accumulators)
            for st in range(ST2):
                v3 = v_sb[st][:].rearrange("p (h x) -> p h x", x=VW)
                nc.gpsimd.memset(v3[:, :, DH:DH + 1], 1.0)

            # ---- emission helpers ----
            def proj_qk(which, r):
                """q/k projection for j-tile r: psum halves + per-partition
                bias add -> qT/kT sbuf."""
                xt, wt, dst, bias, w2 = (
                    (xq, wq, qT_sb, bq_sb, T // 2) if which == "q"
                    else (xk, wk, kT_sb, bk_sb, KC)
                )
                for c2 in range(2):
                    ps = ppsg.tile([128, 512], F32, tag="psg", name=f"ps_{which}{r}")
                    for et in range(ET):
                        nc.tensor.matmul(
                            ps[:, 0:w2],
                            lhsT=wt[et][:, r * 128:(r + 1) * 128],
                            rhs=xt[et][:, c2 * w2:(c2 + 1) * w2],
                            start=(et == 0), stop=(et == ET - 1),
                        )
                    nc.vector.tensor_scalar_add(
                        dst[r][:, c2 * w2:(c2 + 1) * w2], ps[:, 0:w2], bias[:, r:r + 1]
                    )

            def proj_v(st):
                """v projection for s-tile st -> 65-col stripes with bias."""
                ps = ppsg.tile([128, 512], F32, tag="psg", name=f"ps_v{st}")
                for et in range(ET):
                    nc.tensor.matmul(
                        ps[:],
                        lhsT=xv[et][:, st * 128:(st + 1) * 128],
                        rhs=wv[et][:],
                        start=(et == 0), stop=(et == ET - 1),
                    )
                v3 = v_sb[st][:].rearrange("p (h x) -> p h x", x=VW)
                nc.vector.tensor_tensor(
                    v3[:, :, 0:DH],
                    ps[:].rearrange("p (h x) -> p h x", x=DH),
                    bvb_sb[:].rearrange("p (h x) -> p h x", x=DH),
                    mybir.AluOpType.add,
                )

            expT = {}   # (h, st) -> sbuf tile [128, T] bf16

            def scores_exp(h, st):
                """scoresT psum [s-tile, T] (2 banks) -> exp -> bf16 sbuf."""
                r, po_ = h // 2, (h % 2) * DH
                ps_s = ppsc.tile([128, T], F32, tag="sc", name=f"ps_s{h}_{st}")
                for c2 in range(2):
                    nc.tensor.matmul(
                        ps_s[:, c2 * 512:(c2 + 1) * 512],
                        lhsT=kT_sb[r][po_:po_ + DH, st * 128:(st + 1) * 128],
                        rhs=qT_sb[r][po_:po_ + DH, c2 * 512:(c2 + 1) * 512],
                        start=True, stop=True,
                    )
                e = pe.tile([128, T], BF, tag="exp", name=f"e{h}_{st}")
                nc.scalar.activation(
                    e[:], ps_s[:],
                    mybir.ActivationFunctionType.Exp,
                    bias=mb_sb[:, st:st + 1], scale=SCALING,
                )
                expT[(h, st)] = e

            def av_block(r, tt):
                """natural-orientation attn@v for head pair r, t-tile tt:
                psum [t 128, 2*65] stripes -> reciprocal + per-partition
                normalize -> a_nat bf16; then DMA-transpose into aT[r]."""
                ps_o = ppsav.tile([128, 2 * VW], F32, tag="av", name=f"ps_o{r}_{tt}")
                for hh in range(2):
                    h = 2 * r + hh
                    for st in range(ST2):
                        nc.tensor.matmul(
                            ps_o[:, hh * VW:(hh + 1) * VW],
                            lhsT=expT[(h, st)][:, tt * 128:(tt + 1) * 128],
                            rhs=v_sb[st][:, h * VW:(h + 1) * VW],
                            start=(st == 0), stop=(st == ST2 - 1),
                        )
                rec = prc.tile([128, 2], F32, tag="rec", name=f"rec{r}_{tt}")
                nc.vector.reciprocal(rec[:], ps_o[:, DH::VW])
                for hh in range(2):
                    nc.vector.tensor_scalar_mul(
                        an_sb[tt][:, r * 128 + hh * DH:r * 128 + (hh + 1) * DH],
                        ps_o[:, hh * VW:hh * VW + DH],
                        rec[:, hh:hh + 1],
                    )
                nc.sync.dma_start_transpose(
                    aT_sb[r][:, tt * 128:(tt + 1) * 128],
                    an_sb[tt][:, r * 128:(r + 1) * 128],
                )

            def outproj(tt, fine=False):
                """out[t-tile] = sum_r aT[r].T @ woT[r]; psum -> sbuf -> HBM.
                fine=True streams the stores in 128-col pieces (last tile)."""
                for oc in range(2):
                    ps_u = ppsg.tile([128, 512], F32, tag="psg", name=f"ps_u{tt}_{oc}")
                    for r in range(JT):
                        nc.tensor.matmul(
                            ps_u[:],
                            lhsT=aT_sb[r][:, tt * 128:(tt + 1) * 128],
                            rhs=woT_sb[r][:, oc * 512:(oc + 1) * 512],
                            start=(r == 0), stop=(r == JT - 1),
                        )
                    pieces = 4 if fine else 1
                    pw_ = 512 // pieces
                    for pz in range(pieces):
                        ot = po.tile([128, pw_], F32, tag="ot" if not fine else "otf",
                                     name=f"ot{tt}_{oc}_{pz}")
                        nc.vector.tensor_copy(ot[:], ps_u[:, pz * pw_:(pz + 1) * pw_])
                        eng = nc.sync if (oc + pz) % 2 == 0 else nc.scalar
                        eng.dma_start(
                            out=out[tt * 128:(tt + 1) * 128,
                                    oc * 512 + pz * pw_:oc * 512 + (pz + 1) * pw_],
                            in_=ot[:],
                        )

            # ---- pipelined emission ----
            # period r: q/k proj r, then scores+exp of heads 2r/2r+1
            # interleaved with av blocks of pair r-1 (v-proj in period 0).
            for r in range(JT):
                proj_qk("q", r)
                proj_qk("k", r)
                fill = [("v", st) for st in range(ST2)] if r == 0 else \
                       [("av", tt) for tt in range(TT)]
                sc = [("sc", st) for st in range(ST2)]
                # lead with two score tiles, then alternate; leftover fill
                # work runs at the end of the period.
                seq = []
                si, fi = 0, 0
                lead = 2
                while si < len(sc) or fi < len(fill):
                    take_sc = si < len(sc) and (lead > 0 or fi >= len(fill))
                    if take_sc:
                        seq.append(sc[si]); si += 1
                        lead -= 1
                    else:
                        seq.append(fill[fi]); fi += 1
                        lead += 1 if si < len(sc) else 0
                for kind, idx in seq:
                    if kind == "sc":
                        scores_exp(2 * r, idx)
                        scores_exp(2 * r + 1, idx)
                    elif kind == "v":
                        proj_v(idx)
                    else:
                        av_block(r - 1, idx)
            # drain: av of last head pair + output projection per t-tile
            for tt in range(TT):
                av_block(JT - 1, tt)
                outproj(tt, fine=(tt == TT - 1))

    return _split_waits(nc) if split else nc


_NC_CACHE = {}


def _get_nc(s2):
    if s2 not in _NC_CACHE:
        _NC_CACHE[s2] = build_nc(s2=s2)
    return _NC_CACHE[s2]


def make_in_maps(query, key, value, key_padding_mask, local_mask,
                 Wq, bq, Wk, bk, Wv, bv, Wo, bo, s2=SKEEP):
    import ml_dtypes
    f = np.float32
    bf = ml_dtypes.bfloat16
    st2 = s2 // 128
    in_maps = []
    for c in range(8):
        b, g = c // 2, c % 2
        gs = slice(g * G, (g + 1) * G)
        mask = np.asarray((key_padding_mask if g == 0 else local_mask)[b])
        if s2 == S:
            xk_g = np.asarray(key[b]).T
            xv_g = np.asarray(value[b]).T
            mbias = np.where(mask, NEG, 0.0).astype(f)
        else:
            idx = np.nonzero(~mask)[0]
            nk = len(idx)
            xk_g = np.zeros((E, s2), f)
            xk_g[:, :nk] = np.asarray(key[b]).T[:, idx]
            xv_g = np.zeros((E, s2), f)
            xv_g[:, :nk] = np.asarray(value[b]).T[:, idx]
            mbias = np.where(np.arange(s2) < nk, 0.0, NEG).astype(f)
        in_maps.append({
            "xqT": np.ascontiguousarray(np.asarray(query[b]).T, dtype=bf),
            "xkT": np.ascontiguousarray(xk_g, dtype=bf),
            "xvT": np.ascontiguousarray(xv_g, dtype=bf),
            "wqT": np.ascontiguousarray(Wq[gs, :].T, dtype=bf),
            "wkT": np.ascontiguousarray(Wk[gs, :].T, dtype=bf),
            "wvT": np.ascontiguousarray(Wv[gs, :].T, dtype=bf),
            "woT": np.ascontiguousarray(Wo[:, gs].T, dtype=bf),
            "mb": np.ascontiguousarray(mbias.reshape(st2, 128).T),
            "bqc": np.ascontiguousarray(np.asarray(bq)[gs].astype(f).reshape(4, 128).T),
            "bkc": np.ascontiguousarray(np.asarray(bk)[gs].astype(f).reshape(4, 128).T),
            "bvb": np.ascontiguousarray(
                np.broadcast_to(np.asarray(bv)[gs].astype(bf), (128, G))
            ),
        })
    return in_maps


def kernel(query, key, value, key_padding_mask, local_mask,
           Wq, bq, Wk, bk, Wv, bv, Wo, bo, _trace=False, _tmpdir=None):
    from concourse.bass_utils import run_bass_kernel_spmd

    # sparse path unless a pathological mask exceeds the static pad size
    max_kept = max(
        int((~np.asarray(key_padding_mask)).sum(axis=1).max()),
        int((~np.asarray(local_mask)).sum(axis=1).max()),
    )
    s2 = SKEEP if max_kept <= SKEEP else S

    nc = _get_nc(s2)
    in_maps = make_in_maps(query, key, value, key_padding_mask, local_mask,
                           Wq, bq, Wk, bk, Wv, bv, Wo, bo, s2=s2)
    try:
        res = run_bass_kernel_spmd(nc, in_maps, list(range(8)),
                                   trace=_trace, tmpdir=_tmpdir)
    except Exception:
        # transient device/transport failures have been observed on the
        # axon path; one fresh attempt is cheap relative to a hard fail
        res = run_bass_kernel_spmd(nc, in_maps, list(range(8)),
                                   trace=_trace, tmpdir=_tmpdir)
    outs = [np.asarray(r["out"]) for r in res.results]
    full = np.stack([outs[2 * b] + outs[2 * b + 1] for b in range(B)])
    full += np.asarray(bo, dtype=np.float32)
    if _trace:
        kernel._last_exec_time_ns = res.exec_time_ns
        kernel._last_profile = res.profile_json
    return full.astype(np.float32)
